# revision 4
# baseline (speedup 1.0000x reference)
"""Trainium2 Bass kernel for nn_MixtureOfExperts (dense MoE, softmax-gated) — v3.

Strategy: expert-parallel across 8 NeuronCores. Core e receives the full
(transposed) token matrix xT plus expert e's weights, computes
    partial_e = gate[:, e:e+1] * (relu(relu(x@W0e + b0e)@W1e + b1e)@Wfe + bfe)
entirely on-core; the host sums the 8 partials. All matmuls run in fp16 with
fp32 PSUM accumulation.

v2 changes vs v1:
- L1/L2 biases ride the ScalarE activation (per-partition bias AP) instead of
  K=1 ones-matmuls: -512 N=512 matmuls. Bias tiles are DVE-copied out of
  their DMA landing tiles first so the Activation's bias AP is not
  DMA-sourced (walrus AC-descriptor sync-wait-slot overflow otherwise).
- Gate logits computed expert-major ([8, T] psum, stationary = 8-col gw
  chunk, cheap LDWEIGHTS) instead of token-major 8-col moving MMs; softmax
  sum + own-expert numerator recovered token-major with one tiny selector
  matmul exp_s.T @ [ones, e0] per 128-token block.
- L3 loops kc-outer/oc-inner so consecutive MMs share the h2 stationary.
- L3 bias via a DVE add of a host-pre-broadcast bf tile directly into PSUM
  (drops the 128 K=1 ones-matmuls); ph pool deepened to 4 banks.
"""

import numpy as np
from contextlib import ExitStack

import concourse.bass as bass
import concourse.mybir as mybir
import concourse.tile as tile
from concourse import bacc
from concourse.bass import ds, ts
from concourse.bass_utils import run_bass_kernel_spmd

P = 128
F16 = mybir.dt.float16
F32 = mybir.dt.float32

# Full problem dims (hardcoded per contract; kernel.py may not read spec.json)
E, D_IN, D_HID, D_OUT, N_TOK = 8, 1024, 2048, 1024, 8192
T_TOK = 512  # tokens per tile


def emit_moe(ctx, tc, io, d_in, d_hid, d_out, n_tok, n_exp, T):
    """Emit the per-core MoE program. io maps names -> bass.AP (DRAM).

    Inputs (per core): xT [d_in, n_tok] f16 (x transposed), gw [d_in, n_exp]
    f16 (expert columns permuted: own expert first), gb [n_exp, 1] f32,
    w0 [d_in, d_hid] f16, b0c [P, d_hid/P] f32 (b0c[p, mc] = bias[mc*P+p]),
    w1 [d_hid, d_hid] f16, b1c like b0c, wf [d_hid, d_out] f16,
    bf [1, d_out] f16. Output: out [n_tok, d_out] f16.
    """
    nc = tc.nc
    AF = mybir.ActivationFunctionType
    KI, KH = d_in // P, d_hid // P
    S = T // P
    ow = min(512, d_out)
    OC = d_out // ow

    wpool = ctx.enter_context(tc.tile_pool(name="w", bufs=1))
    xpool = ctx.enter_context(tc.tile_pool(name="x", bufs=2))
    hpool = ctx.enter_context(tc.tile_pool(name="h", bufs=1))
    opool = ctx.enter_context(tc.tile_pool(name="o", bufs=4))
    gpool = ctx.enter_context(tc.tile_pool(name="g", bufs=8))
    ph = ctx.enter_context(tc.tile_pool(name="ph", bufs=4, space="PSUM"))
    po = ctx.enter_context(tc.tile_pool(name="po", bufs=2, space="PSUM"))
    pgg = ctx.enter_context(tc.tile_pool(name="pgg", bufs=1, space="PSUM"))
    pst = ctx.enter_context(tc.tile_pool(name="pst", bufs=1, space="PSUM"))

    # Resident weights (fit in SBUF: 16 MB fp16 at full size)
    w0_sb = wpool.tile([P, KI, d_hid], F16)
    nc.sync.dma_start(w0_sb[:], io["w0"].rearrange("(ko p) m -> p ko m", p=P))
    w1_sb = wpool.tile([P, KH, d_hid], F16)
    nc.sync.dma_start(w1_sb[:], io["w1"].rearrange("(ko p) m -> p ko m", p=P))
    wf_sb = wpool.tile([P, KH, d_out], F16)
    nc.sync.dma_start(wf_sb[:], io["wf"].rearrange("(ko p) m -> p ko m", p=P))
    gw_sb = wpool.tile([P, KI, n_exp], F16)
    nc.sync.dma_start(gw_sb[:], io["gw"].rearrange("(ko p) m -> p ko m", p=P))
    # Bias DMA landing tiles. The activation bias/scale APs must NOT source
    # directly from a DMA'd tile (the Activation would pick up a DMA-queue
    # semaphore wait and overflow walrus' AC-descriptor sync-wait slots), so
    # copy them through the DVE into the tiles the activations actually read.
    b0_dma = wpool.tile([P, KH], F32)
    nc.gpsimd.dma_start(b0_dma[:], io["b0c"])
    b1_dma = wpool.tile([P, KH], F32)
    nc.gpsimd.dma_start(b1_dma[:], io["b1c"])
    gb_dma = wpool.tile([n_exp, 1], F32)
    nc.gpsimd.dma_start(gb_dma[:], io["gb"])
    bf_sb = wpool.tile([P, d_out], F16)
    nc.gpsimd.dma_start(bf_sb[:], io["bf"])
    b0v = wpool.tile([P, KH], F32)
    nc.vector.tensor_copy(b0v[:], b0_dma[:])
    b1v = wpool.tile([P, KH], F32)
    nc.vector.tensor_copy(b1v[:], b1_dma[:])
    gbv = wpool.tile([n_exp, 1], F32)
    nc.vector.tensor_copy(gbv[:], gb_dma[:])
    # Selector for the gate: col 0 = ones (softmax denominator), col 1 = e0
    # (own expert's numerator — host permutes gW so own expert is row 0).
    sel_sb = wpool.tile([n_exp, 2], F32)
    nc.vector.memset(sel_sb[:, 0:1], 1.0)
    nc.vector.memset(sel_sb[:, 1:2], 0.0)
    nc.vector.memset(sel_sb[0:1, 1:2], 1.0)

    xT_d = io["xT"].rearrange("(ko p) n -> p ko n", p=P)
    out_d = io["out"]

    for t in range(n_tok // T):
        x_sb = xpool.tile([P, KI, T], F16, tag="x")
        nc.sync.dma_start(x_sb[:], xT_d[:, :, ds(t * T, T)])

        # Gate logits expert-major: [n_exp, T] psum, accumulate over kc.
        pgt = pgg.tile([n_exp, T], F32, tag="pg")
        for kc in range(KI):
            nc.tensor.matmul(
                pgt[:], gw_sb[:, kc, :], x_sb[:, kc, :],
                start=(kc == 0), stop=(kc == KI - 1),
            )
        exp_sb = gpool.tile([n_exp, T], F32, tag="exp", bufs=2)
        nc.scalar.activation(exp_sb[:], pgt[:], AF.Exp, bias=gbv[:])

        # h1T[hid, tok] = relu(W0.T @ xT + b0), bias via activation
        h1_sb = hpool.tile([P, KH, T], F16, tag="h1")
        for mc in range(KH):
            pht = ph.tile([P, T], F32, tag="ph")
            for kc in range(KI):
                nc.tensor.matmul(
                    pht[:], w0_sb[:, kc, ts(mc, P)], x_sb[:, kc, :],
                    start=(kc == 0), stop=(kc == KI - 1),
                )
            nc.scalar.activation(
                h1_sb[:, mc, :], pht[:], AF.Relu, bias=b0v[:, mc : mc + 1]
            )

        # Token-major gate columns: st[:, 0] = sum_e exp, st[:, 1] = exp_own.
        # Emitted here (after L1) so the PE doesn't stall on the Exp ACT.
        gates = []
        for s in range(S):
            st = pst.tile([P, 2], F32, tag="st")
            nc.tensor.matmul(st[:], exp_sb[:, ts(s, P)], sel_sb[:], start=True, stop=True)
            rec = gpool.tile([P, 1], F32, tag="rec", bufs=4)
            nc.vector.reciprocal(rec[:], st[:, 0:1])
            gcol = gpool.tile([P, 1], F32, tag="gcol", bufs=4)
            nc.vector.tensor_mul(out=gcol[:], in0=st[:, 1:2], in1=rec[:])
            gates.append(gcol)

        # h2T[hid, tok] = relu(W1.T @ h1T + b1)
        h2_sb = hpool.tile([P, KH, T], F16, tag="h2")
        for mc in range(KH):
            pht = ph.tile([P, T], F32, tag="ph")
            for kc in range(KH):
                nc.tensor.matmul(
                    pht[:], w1_sb[:, kc, ts(mc, P)], h1_sb[:, kc, :],
                    start=(kc == 0), stop=(kc == KH - 1),
                )
            nc.scalar.activation(
                h2_sb[:, mc, :], pht[:], AF.Relu, bias=b1v[:, mc : mc + 1]
            )

        # o[tok, d_out] = (h2 @ Wf + bf) * gate  (token-major; bias via
        # K=1 ones-matmul; kc-outer so both oc-MMs share the h2 stationary)
        for s in range(S):
            pots = []
            for oc in range(OC):
                pot = po.tile([P, ow], F32, tag="po", name=f"pot{oc}")
                pots.append(pot)
            for kc in range(KH):
                for oc in range(OC):
                    nc.tensor.matmul(
                        pots[oc][:], h2_sb[:, kc, ts(s, P)], wf_sb[:, kc, ts(oc, ow)],
                        start=(kc == 0), stop=(kc == KH - 1),
                    )
            for oc in range(OC):
                nc.vector.tensor_add(
                    out=pots[oc][:], in0=pots[oc][:], in1=bf_sb[:, ts(oc, ow)]
                )
                o_sb = opool.tile([P, ow], F16, tag="o")
                nc.vector.tensor_scalar_mul(o_sb[:], pots[oc][:], gates[s][:])
                nc.sync.dma_start(out_d[ds(t * T + s * P, P), ts(oc, ow)], o_sb[:])


def build(d_in=D_IN, d_hid=D_HID, d_out=D_OUT, n_tok=N_TOK, n_exp=E, T=T_TOK):
    # Bacc (not plain Bass): its compile() runs generate_event_semaphores /
    # move_matmul_waits_to_ldweights, which split multi-waits into standalone
    # instructions — the TPB ISA allows one inline semaphore wait per
    # instruction and walrus rejects BIR that exceeds it.
    nc = bacc.Bacc(None, target_bir_lowering=False)
    io = {
        "xT": nc.dram_tensor("xT", [d_in, n_tok], F16, kind="ExternalInput").ap(),
        "gw": nc.dram_tensor("gw", [d_in, n_exp], F16, kind="ExternalInput").ap(),
        "gb": nc.dram_tensor("gb", [n_exp, 1], F32, kind="ExternalInput").ap(),
        "w0": nc.dram_tensor("w0", [d_in, d_hid], F16, kind="ExternalInput").ap(),
        "b0c": nc.dram_tensor("b0c", [P, d_hid // P], F32, kind="ExternalInput").ap(),
        "w1": nc.dram_tensor("w1", [d_hid, d_hid], F16, kind="ExternalInput").ap(),
        "b1c": nc.dram_tensor("b1c", [P, d_hid // P], F32, kind="ExternalInput").ap(),
        "wf": nc.dram_tensor("wf", [d_hid, d_out], F16, kind="ExternalInput").ap(),
        "bf": nc.dram_tensor("bf", [P, d_out], F16, kind="ExternalInput").ap(),
        "out": nc.dram_tensor("out", [n_tok, d_out], F16, kind="ExternalOutput").ap(),
    }
    with tile.TileContext(nc) as tc:
        with ExitStack() as ctx:
            emit_moe(ctx, tc, io, d_in, d_hid, d_out, n_tok, n_exp, T)
    nc.finalize()
    return nc


def make_in_maps(x, gW, gb, W0, b0, W1, b1, Wf, bf):
    """Host-side sharding/layout prep: one input map per core (= per expert)."""
    f32 = np.float32
    KH = D_HID // P
    xT = np.ascontiguousarray(np.asarray(x, f32).T).astype(np.float16)
    gW = np.asarray(gW, f32)
    gb = np.asarray(gb, f32)
    in_maps = []
    for e in range(E):
        perm = [e] + [i for i in range(E) if i != e]
        in_maps.append(
            dict(
                xT=xT,
                gw=np.ascontiguousarray(gW[:, perm]).astype(np.float16),
                gb=np.ascontiguousarray(gb[perm]).reshape(E, 1).astype(f32),
                w0=np.asarray(W0[e], f32).astype(np.float16),
                b0c=np.ascontiguousarray(
                    np.asarray(b0[e], f32).reshape(KH, P).T
                ),
                w1=np.asarray(W1[e], f32).astype(np.float16),
                b1c=np.ascontiguousarray(
                    np.asarray(b1[e], f32).reshape(KH, P).T
                ),
                wf=np.asarray(Wf[e], f32).astype(np.float16),
                bf=np.ascontiguousarray(
                    np.broadcast_to(
                        np.asarray(bf[e], f32).reshape(1, D_OUT), (P, D_OUT)
                    )
                ).astype(np.float16),
            )
        )
    return in_maps


class _Runner:
    """Compile the Bass program once and execute it on n_cores via PJRT
    (mirrors bass2jax.run_bass_via_pjrt but caches the jitted callable so
    repeated executions don't retrace, enabling device-resident timing)."""

    def __init__(self, nc, n_cores):
        import jax
        from jax.sharding import Mesh, PartitionSpec, NamedSharding
        from jax.experimental.shard_map import shard_map
        from concourse import bass2jax, mybir as mb

        bass2jax.install_neuronx_cc_hook()
        self.jax = jax
        self.n_cores = n_cores

        pid_name = nc.partition_id_tensor.name if nc.partition_id_tensor else None
        in_names, out_names, out_avals, zero_outs = [], [], [], []
        for alloc in nc.m.functions[0].allocations:
            if not isinstance(mb.MemoryLocationSet, type) or not isinstance(
                alloc, mb.MemoryLocationSet
            ):
                continue
            if not alloc.memorylocations:
                continue
            name = alloc.memorylocations[0].name
            if alloc.kind == "ExternalInput":
                if name != pid_name:
                    in_names.append(name)
            elif alloc.kind == "ExternalOutput":
                shape = tuple(alloc.tensor_shape)
                dtype = mb.dt.np(alloc.dtype)
                out_avals.append(jax.core.ShapedArray(shape, dtype))
                out_names.append(name)
                zero_outs.append(np.zeros(shape, dtype))
        self.in_names, self.out_names = in_names, out_names
        n_params = len(in_names)
        partition_name = (
            nc.partition_id_tensor.name if nc.partition_id_tensor else None
        )
        all_in_names = tuple(in_names + out_names)
        if partition_name is not None:
            all_in_names = all_in_names + (partition_name,)

        def _body(*args):
            operands = list(args)
            if partition_name is not None:
                operands.append(bass2jax.partition_id_tensor())
            outs = bass2jax._bass_exec_p.bind(
                *operands,
                out_avals=tuple(out_avals),
                in_names=all_in_names,
                out_names=tuple(out_names),
                lowering_input_output_aliases=(),
                sim_require_finite=True,
                sim_require_nnan=True,
                nc=nc,
            )
            return tuple(outs)

        devices = jax.devices()[:n_cores]
        self.mesh = Mesh(np.asarray(devices), ("core",))
        self.sharding = NamedSharding(self.mesh, PartitionSpec("core"))
        in_specs = (PartitionSpec("core"),) * (n_params + len(out_names))
        out_specs = (PartitionSpec("core"),) * len(out_names)
        self.fn = jax.jit(
            shard_map(
                _body,
                mesh=self.mesh,
                in_specs=in_specs,
                out_specs=out_specs,
                check_rep=False,
            ),
            keep_unused=True,
        )
        self.zero_outs = [
            jax.device_put(
                np.zeros((n_cores * z.shape[0], *z.shape[1:]), z.dtype), self.sharding
            )
            for z in zero_outs
        ]

    def put_inputs(self, in_maps):
        concat = [
            np.concatenate([m[name] for m in in_maps], axis=0)
            for name in self.in_names
        ]
        return [self.jax.device_put(c, self.sharding) for c in concat]

    def __call__(self, dev_inputs):
        return self.fn(*dev_inputs, *self.zero_outs)

    def fetch(self, out_arrs):
        """-> list per core of {name: np.ndarray}"""
        res = []
        for c in range(self.n_cores):
            d = {}
            for i, name in enumerate(self.out_names):
                a = np.asarray(out_arrs[i])
                d[name] = a.reshape(self.n_cores, a.shape[0] // self.n_cores, *a.shape[1:])[c]
            res.append(d)
        return res


_built = None


def _get_runner():
    global _built
    if _built is None:
        _built = _Runner(build(), E)
    return _built


def run(x, gW, gb, W0, b0, W1, b1, Wf, bf, time_iters=0):
    import time as _time

    r = _get_runner()
    in_maps = make_in_maps(x, gW, gb, W0, b0, W1, b1, Wf, bf)
    dev_in = r.put_inputs(in_maps)
    out_arrs = r(dev_in)
    self_jax = r.jax
    self_jax.block_until_ready(out_arrs)

    exec_ns = None
    if time_iters:
        # Warm burst: settles the device clock (HAM) and the dispatch
        # pipeline before the measured loop.
        o = None
        for _ in range(10):
            o = r(dev_in)
        self_jax.block_until_ready(o)
        t0 = _time.perf_counter()
        o = None
        for _ in range(time_iters):
            o = r(dev_in)
        self_jax.block_until_ready(o)
        t1 = _time.perf_counter()
        exec_ns = (t1 - t0) / time_iters * 1e9

    res = r.fetch(out_arrs)
    out = np.zeros((N_TOK, D_OUT), np.float32)
    for d in res:
        out += np.asarray(d["out"], dtype=np.float32)
    return out, exec_ns


def kernel(x, gW, gb, W0, b0, W1, b1, Wf, bf):
    out, _ = run(x, gW, gb, W0, b0, W1, b1, Wf, bf)
    return out
